# revision 7
# baseline (speedup 1.0000x reference)
"""MoE (top-1 routed) Trainium2 kernel.

Routing on host (bitwise-matching the reference's fp32 `x @ Wg + bg`
argmax on CPU); expert e's tokens run on NeuronCore e (expert-parallel,
all-reduce-free).  Device math per core, transposed layout (features on
partitions, tokens on free dim), t2 = tanh(z/2), xn = (1+t2)/2,
q = exp((64/7) xn):

    h^T  = W1^T x^T                  (PE bf16, K=1024)
    sw   = (tanh(h/2) + 1) * h       == 2*swish(h)     (ACT+DVE)
    z^T  = (0.5*proj)^T sw           (PE bf16)
    q^m  = exp(esc_m*(t2+1)), esc_m = 32m/7  (m=1,3 on ACT; q2=q1^2,
           q4=q1*q3, q5=q1*q4, q6=q3^2, q7=q3*q4 bf16 products DVE/GP)
    F    = exp(-8 (1+t2)^2) = exp(-32 xn^2)  (ACT square + exp)
    num  = sum_m cm' q^m             (PE diag-bf16 matmuls)
    out  = (num + c0') * F           (DVE stt per 512-chunk, bf16 out)

Only {Exp, Tanh, Square} share one ACT function table on TRN2
(act_info.json `exp_and_others`); Sigmoid/Silu live in other tables and
every switch costs a 1283ns ACT_TABLE_LOAD, so the whole kernel sticks
to this function set.

Coefficients cm' come from a least-squares refit: the true normalized
RBF weights w_j(xn) = b_j / (sum_i b_i + 1e-6) are refit in the device
basis {exp(-32(xn-m/7)^2)}_m over the observed xn range — exact
normalization folded into an 8x8 host-side matrix (max basis error
~3e-4 vs ~1e-2 for a theta-constant approximation), freeing error
budget for the bf16 output DMA.

Performance structure:
 - warmup sized to the launch window: engines come up ~6us, the first
   DMA payload lands ~10.5us (bulk DMA flow starts ~8.6us no matter
   what); ~12 x 512-wide dummy matmuls ramp the PE DVFS p-state to max
   exactly until mm1's data arrives, instead of baseline's 16 that
   serially delayed mm1 by ~6us,
 - mm1 split in uc-pairs: pair A (uc0,1) runs kc-outer so its xk[kc]
   demand cadence matches DMA arrival order; pair B (uc2,3) runs
   uc-outer on fully-resident x,
 - input DMA spread over 4 engine queues (~145 GB/s per queue observed)
   with pair-A-critical tensors first on each queue,
 - PE order z0 z1 z2 num0 z3 num1 num2 num3 hides elementwise latency;
   products split DVE/GPSIMD with a DVE-heavy tail for the last vc
   (q7 of the last vc comes straight from ACT exp),
 - PSUM: 2x 3-bank mega tiles (h/z) + 2x 1-bank num tiles = 8 banks,
 - output in bf16 (halves output HBM traffic).
"""

import os
from contextlib import ExitStack

import numpy as np

N_TOK, D_IN, U_DIM, E_EXP, B_BAS = 8192, 1024, 512, 8, 8
N_CORES = 8
P = 128

MM_MODE = os.environ.get("MOE_MM_MODE", "bf16")
N_WARM = int(os.environ.get("MOE_WARM", "12"))

_prog_cache = {}


def _basis_consts():
    ks = np.linspace(0.0, 1.0, B_BAS).astype(np.float64)
    a = np.exp(-32.0 * ks * ks)          # b_m = a_m * F * q^m
    esc = 32.0 * ks                      # esc_m = 32*m/7
    return ks, a, esc


def _refit_matrix(xlo=0.22, xhi=0.82, G=4001):
    """R[m, j]: approximate the true normalized RBF weight w_j(x) by
    sum_m R[m, j] * exp(-32 (x - m/7)^2) over x in [xlo, xhi]."""
    ks, _, _ = _basis_consts()
    x = np.linspace(xlo, xhi, G)
    B = np.exp(-32.0 * (x[:, None] - ks[None, :]) ** 2)
    den = B.sum(1) + 1e-6
    Wt = B / den[:, None]
    R, *_ = np.linalg.lstsq(B, Wt, rcond=None)
    return R  # [8 (m), 8 (j)]


def build_program(C, b1_zero):
    import concourse.tile as tile
    from concourse import bacc, mybir

    f32 = mybir.dt.float32
    bf16 = mybir.dt.bfloat16
    add = mybir.AluOpType.add
    mult = mybir.AluOpType.mult
    Tanh = mybir.ActivationFunctionType.Tanh
    Exp = mybir.ActivationFunctionType.Exp
    Square = mybir.ActivationFunctionType.Square

    assert C % P == 0
    # 512-wide bank-aligned chunks (the matmul write granularity)
    chunks = []
    t0 = 0
    while t0 < C:
        chunks.append((t0, min(512, C - t0)))
        t0 += 512

    _, _, esc = _basis_consts()

    nc = bacc.Bacc("TRN2", target_bir_lowering=False, debug=False,
                   num_devices=N_CORES)

    xT = nc.dram_tensor("xT", [D_IN, C], bf16, kind="ExternalInput").ap()
    w1 = nc.dram_tensor("w1", [4, P, 8 * P], bf16, kind="ExternalInput").ap()
    p5 = nc.dram_tensor("p5", [U_DIM, U_DIM], bf16, kind="ExternalInput").ap()
    aux = nc.dram_tensor("aux", [P, 28, P], bf16, kind="ExternalInput").ap()
    cv0 = nc.dram_tensor("cv0", [P, 4], f32, kind="ExternalInput").ap()
    b1h = nc.dram_tensor("b1h", [P, 4], f32, kind="ExternalInput").ap()
    outT = nc.dram_tensor("outT", [U_DIM, C], bf16, kind="ExternalOutput").ap()

    xT_r = xT.rearrange("(kc p) c -> p kc c", p=P)          # [128, 8, C]
    w1_r = w1.rearrange("u p k -> p u k")                   # [128, 4, 1024]
    p5_r = p5.rearrange("(uc p) v -> p uc v", p=P)          # [128, 4, 512]
    outT_r = outT.rearrange("(vc p) c -> p vc c", p=P)      # [128, 4, C]

    with tile.TileContext(nc) as tc, ExitStack() as ctx:
        cpool = ctx.enter_context(tc.tile_pool(name="consts", bufs=1))
        bigps = ctx.enter_context(tc.tile_pool(name="bigps", bufs=2,
                                               space="PSUM"))
        wpool = ctx.enter_context(tc.tile_pool(name="work", bufs=2))
        gpool = ctx.enter_context(tc.tile_pool(name="g", bufs=14))

        # ---- SBUF tiles ----
        w1u = [cpool.tile([P, 8 * P], bf16, tag=f"w1_{uc}", name=f"w1_{uc}")
               for uc in range(4)]
        xk = [cpool.tile([P, C], bf16, tag=f"x{kc}", name=f"x{kc}")
              for kc in range(8)]
        p5sb = cpool.tile([P, 4, U_DIM], bf16, tag="p5")
        auxsb = cpool.tile([P, 28, P], bf16, tag="aux")
        cv0sb = cpool.tile([P, 4], f32, tag="cv0")
        b1sb = cpool.tile([P, 4], f32, tag="b1h")

        # ---- input DMA: 3 paced queues (only SP/ACT/GPSIMD can issue),
        # pair-A criticals first on each queue ----
        # sync:   w1u0, xk0(c0), xk2, xk5, p5, cv0
        # scalar: w1u1, xk0(rest), xk3, xk6, aux
        # gpsimd: b1h, xk1, w1u2, xk4, w1u3, xk7
        nc.sync.dma_start(w1u[0][:], w1_r[:, 0, :])
        nc.scalar.dma_start(w1u[1][:], w1_r[:, 1, :])
        nc.gpsimd.dma_start(b1sb[:], b1h[:])
        nc.sync.dma_start(xk[0][:, 0:512], xT_r[:, 0, 0:512])
        nc.scalar.dma_start(xk[0][:, 512:C], xT_r[:, 0, 512:C])
        nc.gpsimd.dma_start(xk[1][:], xT_r[:, 1, :])
        nc.gpsimd.dma_start(w1u[2][:], w1_r[:, 2, :])
        nc.sync.dma_start(xk[2][:], xT_r[:, 2, :])
        nc.scalar.dma_start(xk[3][:], xT_r[:, 3, :])
        nc.gpsimd.dma_start(xk[4][:], xT_r[:, 4, :])
        nc.sync.dma_start(xk[5][:], xT_r[:, 5, :])
        nc.scalar.dma_start(xk[6][:], xT_r[:, 6, :])
        nc.gpsimd.dma_start(w1u[3][:], w1_r[:, 3, :])
        nc.gpsimd.dma_start(xk[7][:], xT_r[:, 7, :])
        nc.sync.dma_start(p5sb[:], p5_r[:])
        nc.sync.dma_start(cv0sb[:], cv0[:])
        nc.scalar.dma_start(auxsb[:], aux[:])

        npps = bigps

        # bias constants for ACT (esc1, esc3, esc7, 1.0) on the idle
        # gpsimd engine right at program start
        bias_vals = [float(esc[1]), float(esc[3]), float(esc[7]), 1.0]
        bsb = cpool.tile([P, len(bias_vals)], f32, tag="bias")
        for i, v in enumerate(bias_vals):
            nc.gpsimd.memset(bsb[:, i:i + 1], v)
        bias_of = {1: bsb[:, 0:1], 3: bsb[:, 1:2], 7: bsb[:, 2:3]}
        one_b = bsb[:, 3:4]

        # ---- PE warmup: ramp the DVFS p-state during the launch+DMA
        # window (engines up ~6us, first payload ~10.5us) ----
        if N_WARM:
            ones = cpool.tile([P, 512], bf16, tag="ones")
            nc.gpsimd.memset(ones[:], 1.0)
            wps = npps.tile([P, 512], f32, tag="np", name="warm")
            for i in range(N_WARM):
                nc.tensor.matmul(wps[:], lhsT=ones[:, 0:P], rhs=ones[:],
                                 start=(i == 0), stop=(i == N_WARM - 1))

        # ---- mm1 + swish:  sw[uc] [128, C] bf16 ----
        hps = [None] * 4
        sws = [None] * 4

        def emit_swish(uc):
            th = wpool.tile([P, C], f32, tag="th", name=f"th{uc}")
            if b1_zero:
                nc.scalar.activation(th[:], hps[uc][:], Tanh, scale=0.5)
            else:
                nc.scalar.activation(th[:], hps[uc][:], Tanh, scale=0.5,
                                     bias=b1sb[:, uc:uc + 1])
            sw = gpool.tile([P, C], bf16, tag="sw", bufs=4, name=f"sw{uc}")
            if b1_zero:
                nc.vector.scalar_tensor_tensor(
                    sw[:], th[:], 1.0, hps[uc][:], op0=add, op1=mult)
            else:
                y = wpool.tile([P, C], f32, tag="y")
                nc.vector.tensor_scalar(
                    y[:], hps[uc][:], b1sb[:, uc:uc + 1], None, op0=add)
                nc.vector.scalar_tensor_tensor(
                    sw[:], th[:], 1.0, y[:], op0=add, op1=mult)
            sws[uc] = sw

        # pair A (uc 0,1): kc-outer — xk demand matches DMA arrival order
        for uc in (0, 1):
            hps[uc] = bigps.tile([P, C], f32, tag="big", name=f"h{uc}")
        for kc in range(8):
            for uc in (0, 1):
                for (o, TN) in chunks:
                    nc.tensor.matmul(
                        hps[uc][:, o:o + TN],
                        lhsT=w1u[uc][:, kc * P:(kc + 1) * P],
                        rhs=xk[kc][:, o:o + TN],
                        start=(kc == 0), stop=(kc == 7),
                    )
        emit_swish(0)
        emit_swish(1)
        # pair B (uc 2,3): uc-outer — x fully resident by now
        for uc in (2, 3):
            hps[uc] = bigps.tile([P, C], f32, tag="big", name=f"h{uc}")
            for kc in range(8):
                for (o, TN) in chunks:
                    nc.tensor.matmul(
                        hps[uc][:, o:o + TN],
                        lhsT=w1u[uc][:, kc * P:(kc + 1) * P],
                        rhs=xk[kc][:, o:o + TN],
                        start=(kc == 0), stop=(kc == 7),
                    )
            emit_swish(uc)

        # ---- per-vc ----
        def emit_zps(vc):
            zps = bigps.tile([P, C], f32, tag="big", name=f"z{vc}")
            for uc in range(4):
                for (o, TN) in chunks:
                    nc.tensor.matmul(
                        zps[:, o:o + TN],
                        lhsT=p5sb[:, uc, vc * P:(vc + 1) * P],
                        rhs=sws[uc][:, o:o + TN],
                        start=(uc == 0), stop=(uc == 3),
                    )
            return zps

        def emit_elem(vc, zps):
            last = vc == 3
            t2 = wpool.tile([P, C], f32, tag="t2", name=f"t2_{vc}")
            nc.scalar.activation(t2[:], zps[:], Tanh, scale=0.5)
            g = [None] * 8
            ge = [1, 3, 7] if last else [1, 3]
            for j in ge:
                g[j] = gpool.tile([P, C], bf16, tag="g", name=f"g{j}_{vc}")
                nc.scalar.activation(g[j][:], t2[:], Exp,
                                     scale=float(esc[j]), bias=bias_of[j])
            # remaining powers as bf16 products; GPSIMD takes q2/q6 except
            # the last vc, whose tail must not wait on the slow engine
            if last:
                prods = ((2, (1, 1), nc.gpsimd),
                         (6, (3, 3), nc.vector),
                         (4, (1, 3), nc.vector),
                         (5, (1, 4), nc.vector))
            else:
                prods = ((2, (1, 1), nc.gpsimd),
                         (4, (1, 3), nc.vector),
                         (5, (1, 4), nc.vector),
                         (6, (3, 3), nc.gpsimd),
                         (7, (3, 4), nc.vector))
            for j, (ja, jb), eng in prods:
                g[j] = gpool.tile([P, C], bf16, tag="g", name=f"g{j}_{vc}")
                eng.tensor_tensor(g[j][:], g[ja][:], g[jb][:], mult)
            s2 = wpool.tile([P, C], f32, tag="s2", name=f"s2_{vc}")
            nc.scalar.activation(s2[:], t2[:], Square, scale=1.0, bias=one_b)
            F = wpool.tile([P, C], f32, tag="F", name=f"F_{vc}")
            nc.scalar.activation(F[:], s2[:], Exp, scale=-8.0)
            return g, F

        # num j-order by g availability: q1, q3 (ACT), q4 (DVE), q2 (GP),
        # q5, q7 (DVE), q6 (GP last). Last vc: q7 from ACT, q6/q4/q5 DVE.
        J_ORDER = (1, 3, 4, 2, 5, 7, 6)
        J_ORDER_LAST = (1, 3, 7, 2, 6, 4, 5)

        def emit_num_out(vc, g, F):
            jo = J_ORDER_LAST if vc == 3 else J_ORDER
            for ci, (o, TN) in enumerate(chunks):
                nps = npps.tile([P, 512], f32, tag="np", name=f"n{vc}_{ci}")
                for jn, j in enumerate(jo):
                    nc.tensor.matmul(
                        nps[:, :TN],
                        lhsT=auxsb[:, vc * 7 + (j - 1), :],
                        rhs=g[j][:, o:o + TN],
                        start=(jn == 0), stop=(jn == 6),
                    )
                ov = wpool.tile([P, 512], bf16, tag="ov", bufs=3,
                                name=f"ov{vc}_{ci}")
                nc.vector.scalar_tensor_tensor(
                    ov[:, :TN], nps[:, :TN], cv0sb[:, vc:vc + 1],
                    F[:, o:o + TN], op0=add, op1=mult)
                nc.sync.dma_start(outT_r[:, vc, o:o + TN], ov[:, :TN])

        zq = {}
        el = {}
        zq[0] = emit_zps(0)
        el[0] = emit_elem(0, zq[0])
        zq[1] = emit_zps(1)
        el[1] = emit_elem(1, zq[1])
        zq[2] = emit_zps(2)
        el[2] = emit_elem(2, zq[2])
        emit_num_out(0, *el[0])
        zq[3] = emit_zps(3)
        el[3] = emit_elem(3, zq[3])
        emit_num_out(1, *el[1])
        emit_num_out(2, *el[2])
        emit_num_out(3, *el[3])

    nc.compile()
    return nc, chunks


def _get_program(C, mm_mode, b1_zero):
    key = (C, mm_mode, b1_zero)
    if key not in _prog_cache:
        _prog_cache[key] = build_program(C, b1_zero)
    return _prog_cache[key]


def _route_on_host(x, Wg, bg):
    """Expert assignment, bitwise-matching the reference's fp32 CPU math."""
    import jax
    import jax.numpy as jnp

    cpu = jax.devices("cpu")[0]
    with jax.default_device(cpu):
        logits = jnp.asarray(x) @ jnp.asarray(Wg) + jnp.asarray(bg)
        eid = np.asarray(jnp.argmax(logits, axis=-1))
    return eid


def make_in_maps(x, W1, b1, proj, ctrl, scaling, Wg, bg, mm_mode=None):
    import ml_dtypes

    bf = ml_dtypes.bfloat16

    x = np.asarray(x, dtype=np.float32)
    eid = _route_on_host(x, Wg, bg)
    order = np.argsort(eid, kind="stable")
    counts = np.bincount(eid, minlength=E_EXP)
    starts = np.zeros(E_EXP + 1, dtype=np.int64)
    starts[1:] = np.cumsum(counts)
    C = int(max(counts.max(), 1))
    C = ((C + P - 1) // P) * P

    _, a_m, _ = _basis_consts()
    R = _refit_matrix()

    cvf = (np.asarray(ctrl, np.float32)
           * np.asarray(scaling, np.float32)[:, None, :])   # [E, B(j), U]
    # exact-normalization refit + device-basis scaling a_m
    cvs = np.einsum("mj,eju->emu", R, cvf.astype(np.float64))
    cvs = (cvs * a_m[None, :, None]).astype(np.float32)     # [E, B(m), U]
    b1f = np.asarray(b1, np.float32)
    b1_zero = not np.any(b1f)

    in_maps = []
    for e in range(E_EXP):
        idx = order[starts[e]:starts[e + 1]]
        xT = np.zeros((D_IN, C), dtype=bf)
        if len(idx):
            xT[:, :len(idx)] = x[idx].T.astype(bf)
        cv_dev = np.ascontiguousarray(
            cvs[e].T.reshape(4, P, B_BAS).transpose(1, 0, 2))  # [p, vc, m]
        cv0_dev = np.ascontiguousarray(cv_dev[:, :, 0])
        b1h = np.ascontiguousarray(
            (0.5 * b1f[e]).reshape(4, P).T).astype(np.float32)
        # aux[p, vc*7+(m-1), pp] = (pp==p) * cvs[e, m, vc*128+p], m=1..7
        aux = np.zeros((P, 28, P), dtype=bf)
        ar = np.arange(P)
        for vc in range(4):
            for m in range(1, 8):
                aux[ar, vc * 7 + (m - 1), ar] = cv_dev[:, vc, m]
        w1h = np.ascontiguousarray(
            np.asarray(W1[e], np.float32).reshape(8, P, 4, P)
            .transpose(2, 1, 0, 3).reshape(4, P, 8 * P)).astype(bf)
        in_maps.append({
            "xT": xT,
            "w1": w1h,
            "p5": (0.5 * np.asarray(proj[e], np.float32)).astype(bf),
            "aux": aux,
            "cv0": cv0_dev,
            "b1h": b1h,
        })
    return in_maps, order, starts, counts, C, b1_zero


def kernel(x, W1, b1, proj, ctrl, scaling, Wg, bg):
    from concourse.bass_utils import run_bass_kernel_spmd

    in_maps, order, starts, counts, C, b1_zero = make_in_maps(
        x, W1, b1, proj, ctrl, scaling, Wg, bg, MM_MODE)
    nc, _ = _get_program(C, MM_MODE, b1_zero)

    res = run_bass_kernel_spmd(nc, in_maps, list(range(N_CORES)))

    out = np.empty((N_TOK, U_DIM), dtype=np.float32)
    for e in range(E_EXP):
        cnt = int(counts[e])
        if cnt:
            out[order[starts[e]:starts[e + 1]]] = (
                res.results[e]["outT"][:, :cnt].astype(np.float32).T)
    return out


# revision 10
# speedup vs baseline: 1.0723x; 1.0723x over previous
"""MoE (top-1 routed) Trainium2 kernel.

Routing on host (bitwise-matching the reference's fp32 `x @ Wg + bg`
argmax on CPU); expert e's tokens run on NeuronCore e (expert-parallel,
all-reduce-free).  Device math per core, transposed layout (features on
partitions, tokens on free dim), t2 = tanh(z/2), xn = (1+t2)/2,
q = exp((64/7) xn):

    h^T  = W1^T x^T                  (PE bf16, K=1024)
    sw   = (tanh(h/2) + 1) * h       == 2*swish(h)     (ACT+DVE)
    z^T  = (0.5*proj)^T sw           (PE bf16)
    q^m  = exp(esc_m*(t2+1)), esc_m = 32m/7  (m=1,3 on ACT; q2=q1^2,
           q4=q1*q3, q5=q1*q4, q6=q3^2, q7=q3*q4 bf16 products DVE/GP)
    F    = exp(-8 (1+t2)^2) = exp(-32 xn^2)  (ACT square + exp)
    num  = sum_m cm' q^m             (PE diag-bf16 matmuls)
    out  = (num + c0') * F           (DVE stt per 512-chunk, bf16 out)

Only {Exp, Tanh, Square} share one ACT function table on TRN2
(act_info.json `exp_and_others`); Sigmoid/Silu live in other tables and
every switch costs a 1283ns ACT_TABLE_LOAD, so the whole kernel sticks
to this function set.

Coefficients cm' come from a least-squares refit: the true normalized
RBF weights w_j(xn) = b_j / (sum_i b_i + 1e-6) are refit in the device
basis {exp(-32(xn-m/7)^2)}_m over the observed xn range — exact
normalization folded into an 8x8 host-side matrix (max basis error
~3e-4 vs ~1e-2 for a theta-constant approximation), freeing error
budget for the bf16 output DMA.

Performance structure:
 - warmup sized to the launch window: engines come up ~6us, the first
   DMA payload lands ~10.5us (bulk DMA flow starts ~8.6us no matter
   what); ~12 x 512-wide dummy matmuls ramp the PE DVFS p-state to max
   exactly until mm1's data arrives, instead of baseline's 16 that
   serially delayed mm1 by ~6us,
 - mm1 split in uc-pairs: pair A (uc0,1) runs kc-outer so its xk[kc]
   demand cadence matches DMA arrival order; pair B (uc2,3) runs
   uc-outer on fully-resident x,
 - input DMA spread over 4 engine queues (~145 GB/s per queue observed)
   with pair-A-critical tensors first on each queue,
 - PE order z0 z1 z2 num0 z3 num1 num2 num3 hides elementwise latency;
   products split DVE/GPSIMD with a DVE-heavy tail for the last vc
   (q7 of the last vc comes straight from ACT exp),
 - PSUM: 2x 3-bank mega tiles (h/z) + 2x 1-bank num tiles = 8 banks,
 - output in bf16 (halves output HBM traffic).
"""

import os
from contextlib import ExitStack

import numpy as np

N_TOK, D_IN, U_DIM, E_EXP, B_BAS = 8192, 1024, 512, 8, 8
N_CORES = 8
P = 128

MM_MODE = os.environ.get("MOE_MM_MODE", "bf16")
N_WARM = int(os.environ.get("MOE_WARM", "12"))

_prog_cache = {}


def _basis_consts():
    ks = np.linspace(0.0, 1.0, B_BAS).astype(np.float64)
    a = np.exp(-32.0 * ks * ks)          # b_m = a_m * F * q^m
    esc = 32.0 * ks                      # esc_m = 32*m/7
    return ks, a, esc


def _refit_matrix(xlo=0.22, xhi=0.82, G=4001):
    """R[m, j]: approximate the true normalized RBF weight w_j(x) by
    sum_m R[m, j] * exp(-32 (x - m/7)^2) over x in [xlo, xhi]."""
    ks, _, _ = _basis_consts()
    x = np.linspace(xlo, xhi, G)
    B = np.exp(-32.0 * (x[:, None] - ks[None, :]) ** 2)
    den = B.sum(1) + 1e-6
    Wt = B / den[:, None]
    R, *_ = np.linalg.lstsq(B, Wt, rcond=None)
    return R  # [8 (m), 8 (j)]


def build_program(C, b1_zero):
    import concourse.tile as tile
    from concourse import bacc, mybir

    f32 = mybir.dt.float32
    bf16 = mybir.dt.bfloat16
    add = mybir.AluOpType.add
    mult = mybir.AluOpType.mult
    Tanh = mybir.ActivationFunctionType.Tanh
    Exp = mybir.ActivationFunctionType.Exp
    Square = mybir.ActivationFunctionType.Square

    assert C % P == 0
    # 512-wide bank-aligned chunks (the matmul write granularity)
    chunks = []
    t0 = 0
    while t0 < C:
        chunks.append((t0, min(512, C - t0)))
        t0 += 512

    _, _, esc = _basis_consts()

    nc = bacc.Bacc("TRN2", target_bir_lowering=False, debug=False,
                   num_devices=N_CORES)

    xT = nc.dram_tensor("xT", [D_IN, C], bf16, kind="ExternalInput").ap()
    w1 = nc.dram_tensor("w1", [4, P, 8 * P], bf16, kind="ExternalInput").ap()
    p5 = nc.dram_tensor("p5", [U_DIM, U_DIM], bf16, kind="ExternalInput").ap()
    aux = nc.dram_tensor("aux", [P, 28, P], bf16, kind="ExternalInput").ap()
    cv0 = nc.dram_tensor("cv0", [P, 4], f32, kind="ExternalInput").ap()
    b1h = None
    if not b1_zero:
        b1h = nc.dram_tensor("b1h", [P, 4], f32, kind="ExternalInput").ap()
    outT = nc.dram_tensor("outT", [U_DIM, C], bf16, kind="ExternalOutput").ap()

    xT_r = xT.rearrange("(kc p) c -> p kc c", p=P)          # [128, 8, C]
    w1_r = w1.rearrange("u p k -> p u k")                   # [128, 4, 1024]
    p5_r = p5.rearrange("(uc p) v -> p uc v", p=P)          # [128, 4, 512]
    outT_r = outT.rearrange("(vc p) c -> p vc c", p=P)      # [128, 4, C]

    with tile.TileContext(nc) as tc, ExitStack() as ctx:
        cpool = ctx.enter_context(tc.tile_pool(name="consts", bufs=1))
        bigps = ctx.enter_context(tc.tile_pool(name="bigps", bufs=2,
                                               space="PSUM"))
        wpool = ctx.enter_context(tc.tile_pool(name="work", bufs=2))
        gpool = ctx.enter_context(tc.tile_pool(name="g", bufs=14))

        # ---- SBUF tiles ----
        w1u = [cpool.tile([P, 8 * P], bf16, tag=f"w1_{uc}", name=f"w1_{uc}")
               for uc in range(4)]
        xk = [cpool.tile([P, C], bf16, tag=f"x{kc}", name=f"x{kc}")
              for kc in range(8)]
        p5sb = cpool.tile([P, 4, U_DIM], bf16, tag="p5")
        auxsb = cpool.tile([P, 28, P], bf16, tag="aux")
        cv0sb = cpool.tile([P, 4], f32, tag="cv0")
        b1sb = None
        if not b1_zero:
            b1sb = cpool.tile([P, 4], f32, tag="b1h")

        npps = bigps

        # warmup seed + ACT bias constants on the idle DVE engine, first
        # thing, so nothing queues ahead of them
        bias_vals = [float(esc[1]), float(esc[3]), float(esc[7]), 1.0]
        bsb = cpool.tile([P, len(bias_vals)], f32, tag="bias")
        ones = cpool.tile([P, 512], bf16, tag="ones")
        nc.vector.memset(ones[:], 1.0)
        for i, v in enumerate(bias_vals):
            nc.vector.memset(bsb[:, i:i + 1], v)
        bias_of = {1: bsb[:, 0:1], 3: bsb[:, 1:2], 7: bsb[:, 2:3]}
        one_b = bsb[:, 3:4]

        # ---- PE warmup: ramp the DVFS p-state during the launch+DMA
        # window (engines up ~6us, first mm1 payload lands ~11.3us) ----
        if N_WARM:
            wps = npps.tile([P, 512], f32, tag="np", name="warm")
            for i in range(N_WARM):
                nc.tensor.matmul(wps[:], lhsT=ones[:, 0:P], rhs=ones[:],
                                 start=(i == 0), stop=(i == N_WARM - 1))

        # ---- input DMA: sync+scalar carry the paced x/w1 stream in
        # consumption order; gpsimd (expensive ~650ns software issue, but
        # otherwise idle) carries only the late pair-B weights ----
        # sync:   w1u0, xk0(c0), xk0(rest), xk2, xk4, xk6, p5, cv0
        # scalar: w1u1, xk1, xk3, xk5, xk7, aux
        # gpsimd: w1u2, w1u3
        nc.sync.dma_start(w1u[0][:], w1_r[:, 0, :])
        nc.scalar.dma_start(w1u[1][:], w1_r[:, 1, :])
        nc.sync.dma_start(xk[0][:, 0:512], xT_r[:, 0, 0:512])
        nc.gpsimd.dma_start(w1u[2][:], w1_r[:, 2, :])
        nc.gpsimd.dma_start(w1u[3][:], w1_r[:, 3, :])
        nc.sync.dma_start(xk[0][:, 512:C], xT_r[:, 0, 512:C])
        nc.scalar.dma_start(xk[1][:], xT_r[:, 1, :])
        nc.sync.dma_start(xk[2][:], xT_r[:, 2, :])
        nc.scalar.dma_start(xk[3][:], xT_r[:, 3, :])
        nc.sync.dma_start(xk[4][:], xT_r[:, 4, :])
        nc.scalar.dma_start(xk[5][:], xT_r[:, 5, :])
        nc.sync.dma_start(xk[6][:], xT_r[:, 6, :])
        nc.scalar.dma_start(xk[7][:], xT_r[:, 7, :])
        nc.sync.dma_start(p5sb[:], p5_r[:])
        nc.sync.dma_start(cv0sb[:], cv0[:])
        nc.scalar.dma_start(auxsb[:], aux[:])
        if not b1_zero:
            nc.scalar.dma_start(b1sb[:], b1h[:])

        # ---- mm1 + swish:  sw[uc] [128, C] bf16 ----
        hps = [None] * 4
        sws = [None] * 4

        def emit_swish(uc):
            th = wpool.tile([P, C], f32, tag="th", name=f"th{uc}")
            if b1_zero:
                nc.scalar.activation(th[:], hps[uc][:], Tanh, scale=0.5)
            else:
                nc.scalar.activation(th[:], hps[uc][:], Tanh, scale=0.5,
                                     bias=b1sb[:, uc:uc + 1])
            sw = gpool.tile([P, C], bf16, tag="sw", bufs=4, name=f"sw{uc}")
            if b1_zero:
                nc.vector.scalar_tensor_tensor(
                    sw[:], th[:], 1.0, hps[uc][:], op0=add, op1=mult)
            else:
                y = wpool.tile([P, C], f32, tag="y")
                nc.vector.tensor_scalar(
                    y[:], hps[uc][:], b1sb[:, uc:uc + 1], None, op0=add)
                nc.vector.scalar_tensor_tensor(
                    sw[:], th[:], 1.0, y[:], op0=add, op1=mult)
            sws[uc] = sw

        # pair A (uc 0,1): kc-outer — xk demand matches DMA arrival order
        for uc in (0, 1):
            hps[uc] = bigps.tile([P, C], f32, tag="big", name=f"h{uc}")
        for kc in range(8):
            for uc in (0, 1):
                for (o, TN) in chunks:
                    nc.tensor.matmul(
                        hps[uc][:, o:o + TN],
                        lhsT=w1u[uc][:, kc * P:(kc + 1) * P],
                        rhs=xk[kc][:, o:o + TN],
                        start=(kc == 0), stop=(kc == 7),
                    )
        emit_swish(0)
        emit_swish(1)
        # pair B (uc 2,3): uc-outer — x fully resident by now
        for uc in (2, 3):
            hps[uc] = bigps.tile([P, C], f32, tag="big", name=f"h{uc}")
            for kc in range(8):
                for (o, TN) in chunks:
                    nc.tensor.matmul(
                        hps[uc][:, o:o + TN],
                        lhsT=w1u[uc][:, kc * P:(kc + 1) * P],
                        rhs=xk[kc][:, o:o + TN],
                        start=(kc == 0), stop=(kc == 7),
                    )
            emit_swish(uc)

        # ---- per-vc ----
        def emit_zps(vc):
            zps = bigps.tile([P, C], f32, tag="big", name=f"z{vc}")
            for uc in range(4):
                for (o, TN) in chunks:
                    nc.tensor.matmul(
                        zps[:, o:o + TN],
                        lhsT=p5sb[:, uc, vc * P:(vc + 1) * P],
                        rhs=sws[uc][:, o:o + TN],
                        start=(uc == 0), stop=(uc == 3),
                    )
            return zps

        def emit_elem(vc, zps):
            last = vc == 3
            t2 = wpool.tile([P, C], f32, tag="t2", name=f"t2_{vc}")
            nc.scalar.activation(t2[:], zps[:], Tanh, scale=0.5)
            g = [None] * 8
            ge = [1, 3, 7] if last else [1, 3]
            for j in ge:
                g[j] = gpool.tile([P, C], bf16, tag="g", name=f"g{j}_{vc}")
                nc.scalar.activation(g[j][:], t2[:], Exp,
                                     scale=float(esc[j]), bias=bias_of[j])
            # remaining powers as bf16 products; GPSIMD takes q2/q6 except
            # the last vc, whose tail must not wait on the slow engine
            if last:
                prods = ((2, (1, 1), nc.gpsimd),
                         (6, (3, 3), nc.vector),
                         (4, (1, 3), nc.vector),
                         (5, (1, 4), nc.vector))
            else:
                prods = ((2, (1, 1), nc.gpsimd),
                         (4, (1, 3), nc.vector),
                         (5, (1, 4), nc.vector),
                         (6, (3, 3), nc.gpsimd),
                         (7, (3, 4), nc.vector))
            for j, (ja, jb), eng in prods:
                g[j] = gpool.tile([P, C], bf16, tag="g", name=f"g{j}_{vc}")
                eng.tensor_tensor(g[j][:], g[ja][:], g[jb][:], mult)
            s2 = wpool.tile([P, C], f32, tag="s2", name=f"s2_{vc}")
            nc.scalar.activation(s2[:], t2[:], Square, scale=1.0, bias=one_b)
            F = wpool.tile([P, C], f32, tag="F", name=f"F_{vc}")
            nc.scalar.activation(F[:], s2[:], Exp, scale=-8.0)
            return g, F

        # num j-order by g availability: q1, q3 (ACT), q4 (DVE), q2 (GP),
        # q5, q7 (DVE), q6 (GP last). Last vc: q7 from ACT, q6/q4/q5 DVE.
        J_ORDER = (1, 3, 4, 2, 5, 7, 6)
        J_ORDER_LAST = (1, 3, 7, 2, 6, 4, 5)

        def emit_num_out(vc, g, F):
            jo = J_ORDER_LAST if vc == 3 else J_ORDER
            for ci, (o, TN) in enumerate(chunks):
                nps = npps.tile([P, 512], f32, tag="np", name=f"n{vc}_{ci}")
                for jn, j in enumerate(jo):
                    nc.tensor.matmul(
                        nps[:, :TN],
                        lhsT=auxsb[:, vc * 7 + (j - 1), :],
                        rhs=g[j][:, o:o + TN],
                        start=(jn == 0), stop=(jn == 6),
                    )
                ov = wpool.tile([P, 512], bf16, tag="ov", bufs=3,
                                name=f"ov{vc}_{ci}")
                nc.vector.scalar_tensor_tensor(
                    ov[:, :TN], nps[:, :TN], cv0sb[:, vc:vc + 1],
                    F[:, o:o + TN], op0=add, op1=mult)
                nc.sync.dma_start(outT_r[:, vc, o:o + TN], ov[:, :TN])

        zq = {}
        el = {}
        zq[0] = emit_zps(0)
        el[0] = emit_elem(0, zq[0])
        zq[1] = emit_zps(1)
        el[1] = emit_elem(1, zq[1])
        zq[2] = emit_zps(2)
        el[2] = emit_elem(2, zq[2])
        emit_num_out(0, *el[0])
        zq[3] = emit_zps(3)
        el[3] = emit_elem(3, zq[3])
        emit_num_out(1, *el[1])
        emit_num_out(2, *el[2])
        emit_num_out(3, *el[3])

    nc.compile()
    return nc, chunks


def _get_program(C, mm_mode, b1_zero):
    key = (C, mm_mode, b1_zero)
    if key not in _prog_cache:
        _prog_cache[key] = build_program(C, b1_zero)
    return _prog_cache[key]


def _route_on_host(x, Wg, bg):
    """Expert assignment, bitwise-matching the reference's fp32 CPU math."""
    import jax
    import jax.numpy as jnp

    cpu = jax.devices("cpu")[0]
    with jax.default_device(cpu):
        logits = jnp.asarray(x) @ jnp.asarray(Wg) + jnp.asarray(bg)
        eid = np.asarray(jnp.argmax(logits, axis=-1))
    return eid


def make_in_maps(x, W1, b1, proj, ctrl, scaling, Wg, bg, mm_mode=None):
    import ml_dtypes

    bf = ml_dtypes.bfloat16

    x = np.asarray(x, dtype=np.float32)
    eid = _route_on_host(x, Wg, bg)
    order = np.argsort(eid, kind="stable")
    counts = np.bincount(eid, minlength=E_EXP)
    starts = np.zeros(E_EXP + 1, dtype=np.int64)
    starts[1:] = np.cumsum(counts)
    C = int(max(counts.max(), 1))
    C = ((C + P - 1) // P) * P

    _, a_m, _ = _basis_consts()
    R = _refit_matrix()

    cvf = (np.asarray(ctrl, np.float32)
           * np.asarray(scaling, np.float32)[:, None, :])   # [E, B(j), U]
    # exact-normalization refit + device-basis scaling a_m
    cvs = np.einsum("mj,eju->emu", R, cvf.astype(np.float64))
    cvs = (cvs * a_m[None, :, None]).astype(np.float32)     # [E, B(m), U]
    b1f = np.asarray(b1, np.float32)
    b1_zero = not np.any(b1f)

    in_maps = []
    for e in range(E_EXP):
        idx = order[starts[e]:starts[e + 1]]
        xT = np.zeros((D_IN, C), dtype=bf)
        if len(idx):
            xT[:, :len(idx)] = x[idx].T.astype(bf)
        cv_dev = np.ascontiguousarray(
            cvs[e].T.reshape(4, P, B_BAS).transpose(1, 0, 2))  # [p, vc, m]
        cv0_dev = np.ascontiguousarray(cv_dev[:, :, 0])
        b1h = np.ascontiguousarray(
            (0.5 * b1f[e]).reshape(4, P).T).astype(np.float32)
        # aux[p, vc*7+(m-1), pp] = (pp==p) * cvs[e, m, vc*128+p], m=1..7
        aux = np.zeros((P, 28, P), dtype=bf)
        ar = np.arange(P)
        for vc in range(4):
            for m in range(1, 8):
                aux[ar, vc * 7 + (m - 1), ar] = cv_dev[:, vc, m]
        w1h = np.ascontiguousarray(
            np.asarray(W1[e], np.float32).reshape(8, P, 4, P)
            .transpose(2, 1, 0, 3).reshape(4, P, 8 * P)).astype(bf)
        im = {
            "xT": xT,
            "w1": w1h,
            "p5": (0.5 * np.asarray(proj[e], np.float32)).astype(bf),
            "aux": aux,
            "cv0": cv0_dev,
        }
        if not b1_zero:
            im["b1h"] = b1h
        in_maps.append(im)
    return in_maps, order, starts, counts, C, b1_zero


def kernel(x, W1, b1, proj, ctrl, scaling, Wg, bg):
    from concourse.bass_utils import run_bass_kernel_spmd

    in_maps, order, starts, counts, C, b1_zero = make_in_maps(
        x, W1, b1, proj, ctrl, scaling, Wg, bg, MM_MODE)
    nc, _ = _get_program(C, MM_MODE, b1_zero)

    res = run_bass_kernel_spmd(nc, in_maps, list(range(N_CORES)))

    out = np.empty((N_TOK, U_DIM), dtype=np.float32)
    for e in range(E_EXP):
        cnt = int(counts[e])
        if cnt:
            out[order[starts[e]:starts[e + 1]]] = (
                res.results[e]["outT"][:, :cnt].astype(np.float32).T)
    return out
